# revision 23
# baseline (speedup 1.0000x reference)
"""GraphSAGE predictor on 8 Trainium2 NeuronCores (Bass/Tile).

Strategy (data-parallel over dst nodes):
- Layer 1: each core computes 12500 of the 100000 dst rows, 98 tiles of
  128. Per tile, 11 indirect DMAs (10 neighbors + self, one row per
  partition) gather f32 feature rows, casting to fp16 in the DMA (SWDGE
  cast). The neighbor sum is a 5-op in-place DVE tree; self/neighbor
  sums go through fp16 PE transposes + fp16 matmuls (PSUM f32),
  bias+relu fused, h1 stored fp16.
- h1 is AllGathered in fp16, chunk-pipelined behind the layer-1 groups.
- Layer 2 computes, per core, exactly the 5632 h2 rows its cosine head
  needs (40 slot-major "a" tiles + 4 "b" item tiles) in head layout,
  directly into SBUF-resident tiles - no h2 AllGather, no head gather.
  Masked sources use a safe index + a 0/1 mask multiply after bias add.
- Head: cosine similarity per user entirely on-chip.
- All index composition is host-side numpy on int32 index arrays only.
"""
import numpy as np

import concourse.bass as bass
import concourse.bacc as bacc
import concourse.tile as tile
import concourse.mybir as mybir
from concourse.bass_utils import run_bass_kernel_spmd

NCORES = 8
N_ENT = 200000
F = 128
HID = 128
OUT = 64
N0 = 160000
N1 = 100000
N_ITEMS = 4096
N_RATED = 40000
S = 10
B = 4096
R_TOT = 40960

L1_SH = N1 // NCORES          # 12500
L1_T = (L1_SH + 127) // 128   # 98
L1_PAD = L1_T * 128           # 12544
H1_FULL = L1_PAD * NCORES     # 100352
L1_CHUNKS = [24, 24, 24, 24, 2]           # tiles per AllGather chunk (tiny tail)
L1_CH_ROWS = [c * 128 for c in L1_CHUNKS]
L1_CH_R0 = [int(x) for x in np.cumsum([0] + L1_CH_ROWS[:-1])]
L1_CH_BASE = [int(x) for x in np.cumsum([0] + [NCORES * r for r in L1_CH_ROWS[:-1]])]

HD_USERS = B // NCORES        # 512
HD_T = HD_USERS // 128        # 4
L2_TA = S * HD_T              # 40 a-tiles (tid = s*HD_T + ut)
L2_T = L2_TA + HD_T           # 44

NQ = 1                        # SWDGE queues (multi-queue probed slower)

_compiled = None


def _build():
    dt = mybir.dt
    nc = bacc.Bacc("TRN2", target_bir_lowering=False, debug=False,
                   num_devices=NCORES, num_swdge_queues=NQ)

    feat = nc.dram_tensor("feat", [N_ENT, F], dt.float32, kind="ExternalInput").ap()
    w1s = nc.dram_tensor("w1s", [F, HID], dt.float16, kind="ExternalInput").ap()
    w1n = nc.dram_tensor("w1n", [F, HID], dt.float16, kind="ExternalInput").ap()
    b1 = nc.dram_tensor("b1", [128, HID], dt.float32, kind="ExternalInput").ap()
    w2s = nc.dram_tensor("w2s", [HID, OUT], dt.float16, kind="ExternalInput").ap()
    w2n = nc.dram_tensor("w2n", [HID, OUT], dt.float16, kind="ExternalInput").ap()
    b2 = nc.dram_tensor("b2", [128, OUT], dt.float32, kind="ExternalInput").ap()
    idn = nc.dram_tensor("idn", [128, 128], dt.float16, kind="ExternalInput").ap()

    l1i = nc.dram_tensor("l1i", [128, L1_T * (S + 1)], dt.int32, kind="ExternalInput").ap()
    l2i = nc.dram_tensor("l2i", [128, L2_T * (S + 1)], dt.int32, kind="ExternalInput").ap()
    msk = nc.dram_tensor("msk", [128, L2_T], dt.float32, kind="ExternalInput").ap()

    pred = nc.dram_tensor("pred", [HD_T, 128], dt.float32, kind="ExternalOutput").ap()

    h1_loc = nc.dram_tensor("h1_loc", [L1_PAD, HID], dt.float16).ap()
    h1_full = nc.dram_tensor("h1_full", [H1_FULL, HID], dt.float16).ap()

    groups = [list(range(NCORES))]
    qctr = [0]

    with tile.TileContext(nc, pool_alloc_mode="queue") as tc:
        with (
            tc.tile_pool(name="const", bufs=1) as cpool,
            tc.tile_pool(name="work", bufs=6) as wpool,
            tc.tile_pool(name="psum", bufs=3, space="PSUM") as ppool,
            tc.tile_pool(name="psumo", bufs=2, space="PSUM") as ppool_o,
        ):
            l1i_t = cpool.tile([128, L1_T * (S + 1)], dt.int32)
            nc.sync.dma_start(l1i_t[:, 0:33], l1i[:, 0:33])
            nc.sync.dma_start(l1i_t[:, 33:], l1i[:, 33:])
            ident = cpool.tile([128, 128], dt.float16)
            nc.sync.dma_start(ident[:], idn)
            w1s_t = cpool.tile([F, HID], dt.float16)
            nc.sync.dma_start(w1s_t[:], w1s)
            w1n_t = cpool.tile([F, HID], dt.float16)
            nc.sync.dma_start(w1n_t[:], w1n)
            b1_t = cpool.tile([128, HID], dt.float32)
            nc.sync.dma_start(b1_t[:], b1)
            w2s_t = cpool.tile([HID, OUT], dt.float16)
            nc.sync.dma_start(w2s_t[:], w2s)
            w2n_t = cpool.tile([HID, OUT], dt.float16)
            nc.sync.dma_start(w2n_t[:], w2n)
            b2_t = cpool.tile([128, OUT], dt.float32)
            nc.sync.dma_start(b2_t[:], b2)
            msk_t = cpool.tile([128, L2_T], dt.float32)
            nc.sync.dma_start(msk_t[:], msk)
            l2i_t = cpool.tile([128, L2_T * (S + 1)], dt.int32)
            nc.sync.dma_start(l2i_t[:], l2i)

            h2a = cpool.tile([128, L2_TA * OUT], dt.float32)
            h2b = cpool.tile([128, HD_T * OUT], dt.float32)

            h1v = h1_loc.rearrange("(t p) f -> t p f", p=128)

            def gather(out_sl, src, idx_col):
                inst = nc.gpsimd.indirect_dma_start(
                    out=out_sl, out_offset=None, in_=src,
                    in_offset=bass.IndirectOffsetOnAxis(ap=idx_col, axis=0),
                )
                q = qctr[0] % NQ
                qctr[0] += 1
                if q:
                    inst.ins.queue = f"qPoolDynamic{q}"
                return inst

            def sage_tile(t, idx_t, src, wself, wnbr, bias_t, d_out, out_sl, mk):
                SP = S + 1
                wide = wpool.tile([128, SP * F], dt.float16, tag="wide", name="wide")
                for j in range(SP):
                    c = t * SP + j
                    gather(wide[:, j * F:(j + 1) * F], src, idx_t[:, c:c + 1])
                # in-place tree sum of neighbor blocks 0..9 -> block 0
                nc.vector.tensor_tensor(out=wide[:, 0:4*F], in0=wide[:, 0:4*F],
                                        in1=wide[:, 4*F:8*F], op=mybir.AluOpType.add)
                nc.vector.tensor_tensor(out=wide[:, 0:2*F], in0=wide[:, 0:2*F],
                                        in1=wide[:, 2*F:4*F], op=mybir.AluOpType.add)
                nc.vector.tensor_tensor(out=wide[:, 0:F], in0=wide[:, 0:F],
                                        in1=wide[:, F:2*F], op=mybir.AluOpType.add)
                nc.vector.tensor_tensor(out=wide[:, 0:F], in0=wide[:, 0:F],
                                        in1=wide[:, 8*F:9*F], op=mybir.AluOpType.add)
                nc.vector.tensor_tensor(out=wide[:, 0:F], in0=wide[:, 0:F],
                                        in1=wide[:, 9*F:10*F], op=mybir.AluOpType.add)
                pnb = ppool.tile([128, 128], dt.float16, tag="pnb", name="pnb")
                nc.tensor.transpose(pnb[:], wide[:, 0:F], ident[:])
                psf = ppool.tile([128, 128], dt.float16, tag="psf", name="psf")
                nc.tensor.transpose(psf[:], wide[:, 10*F:11*F], ident[:])
                nbT = wpool.tile([128, 128], dt.float16, tag="nbT", name="nbT")
                nc.vector.tensor_copy(nbT[:], pnb[:])
                sfT = wpool.tile([128, 128], dt.float16, tag="sfT", name="sfT")
                nc.vector.tensor_copy(sfT[:], psf[:])
                pout = ppool_o.tile([128, d_out], dt.float32, tag="pout", name="pout")
                nc.tensor.matmul(pout[:], lhsT=sfT[:], rhs=wself[:], start=True, stop=False)
                nc.tensor.matmul(pout[:], lhsT=nbT[:], rhs=wnbr[:], start=False, stop=True)
                if out_sl is None:
                    hout = wpool.tile([128, d_out], dt.float16, tag="hout", name="hout")
                    nc.vector.tensor_tensor(out=hout[:], in0=pout[:],
                                            in1=bias_t[:, :d_out],
                                            op=mybir.AluOpType.add)
                    nc.scalar.activation(hout[:], hout[:],
                                         mybir.ActivationFunctionType.Relu)
                    nc.sync.dma_start(h1v[t], hout[:])
                else:
                    nc.vector.tensor_tensor(out=out_sl, in0=pout[:],
                                            in1=bias_t[:, :d_out],
                                            op=mybir.AluOpType.add)
                    if mk is not None:
                        nc.vector.tensor_tensor(
                            out=out_sl, in0=out_sl,
                            in1=mk.to_broadcast([128, d_out]),
                            op=mybir.AluOpType.mult)

            def ag_chunk(g):
                r0, rows, base = L1_CH_R0[g], L1_CH_ROWS[g], L1_CH_BASE[g]
                nc.gpsimd.collective_compute(
                    "AllGather", mybir.AluOpType.bypass, replica_groups=groups,
                    ins=[h1_loc[r0:r0 + rows].opt()],
                    outs=[h1_full[base:base + NCORES * rows].opt()],
                )

            # AG0-AG2 trigger 3 tiles into the next chunk so their h1-write
            # waits are absorbed by gather issue; AG3/AG4 stay in place so the
            # collective cascade still finishes right behind the last tiles.
            t = 0
            pend = None
            for g, ntiles in enumerate(L1_CHUNKS):
                for i in range(ntiles):
                    sage_tile(t, l1i_t, feat, w1s_t, w1n_t, b1_t, HID, None, None)
                    t += 1
                    if i == 8 and pend is not None:
                        ag_chunk(pend)
                        pend = None
                if g < 3:
                    pend = g
                else:
                    if pend is not None:
                        ag_chunk(pend)
                        pend = None
                    ag_chunk(g)

            for ut in range(HD_T):
                # the 11 L2 tiles feeding head group ut, then the head itself
                # (hides head DVE work under the remaining L2 gathers)
                for s in range(S):
                    tid = s * HD_T + ut
                    sage_tile(tid, l2i_t, h1_full, w2s_t, w2n_t, b2_t, OUT,
                              h2a[:, tid * OUT:(tid + 1) * OUT],
                              msk_t[:, tid:tid + 1])
                sage_tile(L2_TA + ut, l2i_t, h1_full, w2s_t, w2n_t, b2_t, OUT,
                          h2b[:, ut * OUT:(ut + 1) * OUT], None)

                bt = h2b[:, ut * OUT:(ut + 1) * OUT]
                ab = wpool.tile([128, S * OUT], dt.float32, tag="ab", name="ab")
                for s in range(S):
                    col = (s * HD_T + ut) * OUT
                    nc.vector.tensor_tensor(
                        out=ab[:, s * OUT:(s + 1) * OUT],
                        in0=h2a[:, col:col + OUT], in1=bt,
                        op=mybir.AluOpType.mult)
                dots = wpool.tile([128, S], dt.float32, tag="dots", name="dots")
                nc.vector.tensor_reduce(
                    out=dots[:], in_=ab[:].rearrange("p (s d) -> p s d", d=OUT),
                    axis=mybir.AxisListType.X, op=mybir.AluOpType.add)
                for s in range(S):
                    col = (s * HD_T + ut) * OUT
                    nc.vector.tensor_tensor(
                        out=ab[:, s * OUT:(s + 1) * OUT],
                        in0=h2a[:, col:col + OUT], in1=h2a[:, col:col + OUT],
                        op=mybir.AluOpType.mult)
                na2 = wpool.tile([128, S], dt.float32, tag="na2", name="na2")
                nc.vector.tensor_reduce(
                    out=na2[:], in_=ab[:].rearrange("p (s d) -> p s d", d=OUT),
                    axis=mybir.AxisListType.X, op=mybir.AluOpType.add)
                bb = wpool.tile([128, OUT], dt.float32, tag="bb", name="bb")
                nc.vector.tensor_tensor(out=bb[:], in0=bt, in1=bt,
                                        op=mybir.AluOpType.mult)
                nb2 = wpool.tile([128, 1], dt.float32, tag="nb2", name="nb2")
                nc.vector.tensor_reduce(
                    out=nb2[:], in_=bb[:], axis=mybir.AxisListType.X,
                    op=mybir.AluOpType.add)
                na = wpool.tile([128, S], dt.float32, tag="na", name="na")
                nc.scalar.activation(na[:], na2[:], mybir.ActivationFunctionType.Sqrt)
                nc.vector.tensor_scalar_max(na[:], na[:], 1e-6)
                nb = wpool.tile([128, 1], dt.float32, tag="nb", name="nb")
                nc.scalar.activation(nb[:], nb2[:], mybir.ActivationFunctionType.Sqrt)
                nc.vector.tensor_scalar_max(nb[:], nb[:], 1e-6)
                den = wpool.tile([128, S], dt.float32, tag="den", name="den")
                nc.vector.tensor_tensor(
                    out=den[:], in0=na[:], in1=nb[:].to_broadcast([128, S]),
                    op=mybir.AluOpType.mult)
                rden = wpool.tile([128, S], dt.float32, tag="rden", name="rden")
                nc.vector.reciprocal(rden[:], den[:])
                sim = wpool.tile([128, S], dt.float32, tag="sim", name="sim")
                nc.vector.tensor_tensor(out=sim[:], in0=dots[:], in1=rden[:],
                                        op=mybir.AluOpType.mult)
                pr = wpool.tile([128, 1], dt.float32, tag="pr", name="pr")
                nc.vector.tensor_reduce(
                    out=pr[:], in_=sim[:], axis=mybir.AxisListType.X,
                    op=mybir.AluOpType.add)
                nc.sync.dma_start(pred[ut], pr[:].rearrange("p o -> (p o)"))

    nc.compile()
    return nc


_L1_TAB = None


def _pad_map_l1(g):
    """Global h1 row (0..N1-1) -> row in the chunk-gathered h1_full layout."""
    global _L1_TAB
    if _L1_TAB is None:
        gl = np.arange(L1_SH)
        tau = gl // 128
        tile_start = np.cumsum([0] + L1_CHUNKS[:-1])
        c = np.searchsorted(tile_start, tau, side="right") - 1
        off = gl - np.asarray(L1_CH_R0)[c]
        _L1_TAB = (np.asarray(L1_CH_BASE)[c], np.asarray(L1_CH_ROWS)[c], off)
    k = g // L1_SH
    gl = g % L1_SH
    base, rows, off = _L1_TAB
    return base[gl] + k * rows[gl] + off[gl]


def _tileize(a, ncols):
    """[T*128 rows, ncols] -> [128, T*ncols] partition-major tile layout."""
    T = a.shape[0] // 128
    return np.ascontiguousarray(
        a.reshape(T, 128, ncols).transpose(1, 0, 2).reshape(128, T * ncols)
    ).astype(np.int32)


def kernel(features, Wself1, Wnbr1, b1, Wself2, Wnbr2, b2,
           input_nodes, nbr1, nbr2, inverse_all, source, item_rep_idx,
           n_items, n_masked):
    global _compiled
    if _compiled is None:
        _compiled = _build()
    nc = _compiled

    features = np.asarray(features, dtype=np.float32)
    input_nodes = np.asarray(input_nodes, dtype=np.int64)
    nbr1 = np.asarray(nbr1, dtype=np.int64)
    nbr2 = np.asarray(nbr2, dtype=np.int64)
    inverse_all = np.asarray(inverse_all, dtype=np.int64)
    source = np.asarray(source, dtype=np.int64)
    item_rep_idx = np.asarray(item_rep_idx, dtype=np.int64)

    scale = np.float32(1.0 / S)
    common = {
        "feat": features,
        "w1s": np.asarray(Wself1, np.float32).astype(np.float16),
        "w1n": (np.asarray(Wnbr1, np.float32) * scale).astype(np.float16),
        "b1": np.tile(np.asarray(b1, np.float32).reshape(1, HID), (128, 1)),
        "w2s": np.asarray(Wself2, np.float32).astype(np.float16),
        "w2n": (np.asarray(Wnbr2, np.float32) * scale).astype(np.float16),
        "b2": np.tile(np.asarray(b2, np.float32).reshape(1, OUT), (128, 1)),
        "idn": np.eye(128, dtype=np.float16),
    }

    src_mat = source.reshape(B, S)
    items = item_rep_idx.reshape(B, S)[:, 0]

    in_maps = []
    for k in range(NCORES):
        # ---- layer 1 indices (into features): [12544, 11] ----
        d0 = k * L1_SH
        d = np.arange(L1_PAD) + d0
        d_c = np.where(d < d0 + L1_SH, d, d0)  # clamp padding to a real row
        l1_idx = np.concatenate(
            [input_nodes[nbr1[d_c]], input_nodes[d_c][:, None]], axis=1)

        # ---- layer 2 dst list in head layout: 40 a-tiles + 4 b-tiles ----
        u = np.arange(HD_USERS) + k * HD_USERS
        srcs = src_mat[u]                       # [512, S]
        masked = srcs < n_masked
        a_row = n_items + (srcs - n_masked)
        q_a = inverse_all[np.where(masked, 0, a_row)]  # [512, S]
        q_b = inverse_all[items[u]]                    # [512]
        dst_q = np.empty(L2_T * 128, dtype=np.int64)
        mask_t = np.ones((128, L2_T), dtype=np.float32)
        for s in range(S):
            for ut in range(HD_T):
                tid = s * HD_T + ut
                rows = slice(ut * 128, (ut + 1) * 128)
                dst_q[tid * 128:(tid + 1) * 128] = q_a[rows, s]
                mask_t[:, tid] = np.where(masked[rows, s], 0.0, 1.0)
        for ut in range(HD_T):
            tid = L2_TA + ut
            dst_q[tid * 128:(tid + 1) * 128] = q_b[ut * 128:(ut + 1) * 128]
        l2_idx = np.concatenate(
            [_pad_map_l1(nbr2[dst_q]), _pad_map_l1(dst_q)[:, None]], axis=1)

        in_maps.append({
            **common,
            "l1i": _tileize(l1_idx, S + 1),
            "l2i": _tileize(l2_idx, S + 1),
            "msk": mask_t,
        })

    res = run_bass_kernel_spmd(nc, in_maps, core_ids=list(range(NCORES)))
    pred = np.concatenate(
        [res.results[k]["pred"].reshape(-1) for k in range(NCORES)]
    )
    return pred.astype(np.float32)
